# revision 1
# baseline (speedup 1.0000x reference)
"""Trainium2 Bass kernel: Atom2Residue (gnn_message_passing).

Math: out[n,c,o] = sum_i fuse[n,c,i] * w[l(c),o,i]  (+ b[o] at c==0)
where fuse[n,c,:] = concat(CA_atom[n,c,:16], res_emb[n,c,:32]), l(c)=floor(sqrt(c)).

Strategy (8 cores, data parallel over residues, no collectives):
  - 18750 residues/core, padded to 18944 = 37 tiles x 512 rows.
  - Device DMAs natural-layout rows, PE-transposes 128x128 chunks to get
    channels-on-partitions, runs 7 matmuls per 512-row tile (block-diagonal, up to 4 coefficients per matmul)
    with weights stationary (matmul cost ~ N stream columns, so coefficients
    are packed 4-per-matmul along K), PE-transposes the [32,n]
    per-coefficient outputs back to natural [n,288] layout.
  - SBUF channel order [res 288 | pad 32 | ca 144] aligns every matmul
    K-window to a legal 0/32/64/96 partition base; each PSUM bank has
    exactly one start=True matmul covering its whole used region.
"""

import os
import sys

for _p in ("/opt/trn_rl_repo",):
    if os.path.isdir(_p) and _p not in sys.path:
        sys.path.insert(0, _p)

import numpy as np

from concourse import bacc, bass, mybir  # noqa: F401
from concourse.bass_utils import run_bass_kernel_spmd
from concourse.tile import TileContext

F32 = mybir.dt.float32

NUM_COEF, ATOM_C, NODE_C = 9, 16, 32
L_OF_COEF = np.floor(np.sqrt(np.arange(NUM_COEF))).astype(np.int64)

N_CORES = 8
R_TOTAL = 150_000
RS = R_TOTAL // N_CORES      # 18750 residues per core
TILE = 512                   # rows per iteration
NSUB, SUB = 4, 128           # 4 sub-tiles of 128 partitions
NIT = -(-RS // TILE)         # 37
RP = NIT * TILE              # 18944 padded rows per core

RES_W = NODE_C * NUM_COEF    # 288
CA_W = ATOM_C * NUM_COEF     # 144
CA0 = RES_W + 32             # 320: atom channels start 64-aligned mod 128
FUSE_W = CA0 + CA_W          # 464 (cols 288:320 are dead padding)
OUT_W = NODE_C * NUM_COEF    # 288


def _mm_schedule():
    """Matmul schedule per tile. SBUF fuse channel k: res j of c at k=32c+j,
    CA i of c at k=288+16c+i. Transposed chunks q: chans [128q, 128q+128).
    PSUM tiles: A holds outT for c0..3 (partitions 32c), B c4..7, C c8."""
    mms = []
    # One K=128 block-diagonal matmul handles res coefficients c..c+3 of a
    # whole transposed chunk (cost of a matmul ~ N stream columns, K-free).
    # It is each bank's unique start=True writer and covers the bank's whole
    # used region, so accumulation is well-defined under both per-element
    # has_written and whole-zero-region PSUM models.
    mms.append(dict(q=0, base=0, K=128, psum="A", ob=0, M=128,
                    start=True, stop=False, kind=("resq", 0)))
    mms.append(dict(q=1, base=0, K=128, psum="B", ob=0, M=128,
                    start=True, stop=False, kind=("resq", 1)))
    mms.append(dict(q=2, base=0, K=32, psum="C", ob=0, M=32,
                    start=True, stop=False, kind=("res", 8)))
    # bias accumulate: out[:, 0, :] += b via rank-1 (b x ones)
    mms.append(dict(q=None, base=0, K=1, psum="A", ob=0, M=32,
                    start=False, stop=False, kind=("bias",)))
    # atom quads (c=4t..4t+3): K=64 block-diagonal lhsT, M=128
    for t in range(2):
        q, base = divmod(CA0 + 64 * t, 128)
        mms.append(dict(q=q, base=base, K=64, psum="AB"[t], ob=0, M=128,
                        start=False, stop=True, kind=("atomq", t)))
    # atom c8: K=16
    q, base = divmod(CA0 + 128, 128)
    mms.append(dict(q=q, base=base, K=16, psum="C", ob=0, M=32,
                    start=False, stop=True, kind=("atom8",)))
    # per-matmul stationary-weight column block in the wsb image
    col = 0
    for mm in mms:
        mm["wcol"] = col
        col += mm["M"]
    return mms, col


MMS, WSB_COLS = _mm_schedule()
IDENT_COL = WSB_COLS            # [128,128] identity block
ONES_COL = IDENT_COL + 128      # ones row (partition 0) [1, TILE]
WSB_FULL = ONES_COL + TILE


def build_wsb(w, b):
    """Stationary-weight SBUF image [128, 64*len(MMS)] mirroring MMS."""
    w = np.asarray(w, np.float32)
    b = np.asarray(b, np.float32)
    wsb = np.zeros((128, WSB_FULL), np.float32)
    wsb[:, IDENT_COL:IDENT_COL + 128] = np.eye(128, dtype=np.float32)
    wsb[0, ONES_COL:ONES_COL + TILE] = 1.0
    for mm in MMS:
        col = mm["wcol"]
        base = mm["base"]
        kind = mm["kind"]
        if kind[0] == "res":
            c = kind[1]
            wsb[base:base + 32, col:col + 32] = w[L_OF_COEF[c]][:, 16:48].T
        elif kind[0] == "resq":
            for cl in range(4):
                c = 4 * kind[1] + cl
                wsb[32 * cl:32 * cl + 32, col + 32 * cl:col + 32 * cl + 32] = \
                    w[L_OF_COEF[c]][:, 16:48].T
        elif kind[0] == "bias":
            wsb[0, col:col + 32] = b
        elif kind[0] == "atomq":
            for cl in range(4):
                c = 4 * kind[1] + cl
                wsb[base + 16 * cl:base + 16 * cl + 16,
                    col + 32 * cl:col + 32 * cl + 32] = \
                    w[L_OF_COEF[c]][:, 0:16].T
        elif kind[0] == "atom8":
            wsb[base:base + 16, col:col + 32] = w[2][:, 0:16].T
        # "zero": leave the block at 0
    return wsb


def build_nc(nit=NIT):
    nc = bacc.Bacc()
    rp = nit * TILE
    res_d = nc.declare_dram_parameter("res", [rp, RES_W], F32, isOutput=False)
    vat_d = nc.declare_dram_parameter("vat", [4 * rp, CA_W], F32, isOutput=False)
    wsb_d = nc.declare_dram_parameter("wsb", [128, WSB_FULL], F32, isOutput=False)
    out_d = nc.declare_dram_parameter("out", [rp, OUT_W], F32, isOutput=True)

    with TileContext(nc) as tc:
        with (
            tc.tile_pool(name="const", bufs=1) as cpool,
            tc.tile_pool(name="fuse", bufs=4) as fuse_pool,
            tc.tile_pool(name="fT", bufs=3) as fT_pool,
            tc.tile_pool(name="outT", bufs=2) as outT_pool,
            tc.tile_pool(name="osb", bufs=4) as osb_pool,
            tc.tile_pool(name="pT", bufs=2, space="PSUM") as pT_pool,
            tc.tile_pool(name="pMM", bufs=1, space="PSUM") as pMM_pool,
            tc.tile_pool(name="pN", bufs=3, space="PSUM") as pN_pool,
        ):
            wsb_sb = cpool.tile([128, WSB_FULL], F32)
            nc.sync.dma_start(out=wsb_sb[:], in_=wsb_d[:])
            ident = wsb_sb[:, IDENT_COL:IDENT_COL + 128]
            ones = wsb_sb[0:1, ONES_COL:ONES_COL + TILE]

            res_r = res_d[:].rearrange("(t s p) j -> t s p j", s=NSUB, p=SUB)
            vat_r = vat_d[:].rearrange("(t s p f) i -> t s p f i",
                                       s=NSUB, p=SUB, f=4)
            out_r = out_d[:].rearrange("(t s p) j -> t s p j", s=NSUB, p=SUB)

            for t in range(nit):
                fuse = fuse_pool.tile([128, NSUB * FUSE_W], F32, tag="fuse")
                fuse_v = fuse[:].rearrange("p (s j) -> p s j", s=NSUB)
                nc.sync.dma_start(
                    out=fuse_v[:, :, 0:RES_W],
                    in_=res_r[t].rearrange("s p j -> p s j"),
                )
                nc.sync.dma_start(
                    out=fuse_v[:, :, CA0:FUSE_W],
                    in_=vat_r[t, :, :, 1, :].rearrange("s p i -> p s i"),
                )
                nc.vector.memset(fuse_v[:, :, RES_W:CA0], 0.0)

                # channels-on-partitions: fT cols = 512*s + 128*q + n
                fT = fT_pool.tile([128, NSUB * TILE], F32, tag="fT")
                for s in range(NSUB):
                    pT = pT_pool.tile([128, TILE], F32, tag="pT")
                    for q in range(4):
                        wch = 128 if q < 3 else FUSE_W - 384  # 80
                        nc.tensor.transpose(
                            pT[0:wch, 128 * q:128 * q + 128],
                            fuse_v[:, s, 128 * q:128 * q + wch],
                            ident,
                        )
                    nc.vector.tensor_copy(fT[:, TILE * s:TILE * s + 384],
                                          pT[:, 0:384])
                    nc.vector.tensor_copy(
                        fT[0:FUSE_W - 384, TILE * s + 384:TILE * (s + 1)],
                        pT[0:FUSE_W - 384, 384:512])

                pA = pMM_pool.tile([128, TILE], F32, tag="pA")
                pB = pMM_pool.tile([128, TILE], F32, tag="pB")
                pC = pMM_pool.tile([32, TILE], F32, tag="pC")
                psums = {"A": pA, "B": pB, "C": pC}
                fT_v = fT[:].rearrange("p (s q n) -> p s q n", s=NSUB, q=4)
                for mm in MMS:
                    lhsT = wsb_sb[mm["base"]:mm["base"] + mm["K"],
                                  mm["wcol"]:mm["wcol"] + mm["M"]]
                    if mm["q"] is None:
                        rhs = ones
                    else:
                        rhs = fT_v[mm["base"]:mm["base"] + mm["K"], :, mm["q"], :]
                    out_ap = psums[mm["psum"]][mm["ob"]:mm["ob"] + mm["M"], :]
                    nc.tensor.matmul(out_ap, lhsT, rhs,
                                     start=mm["start"], stop=mm["stop"],
                                     skip_group_check=True,
                                     tile_position=(mm["base"], mm["ob"]))

                outT0 = outT_pool.tile([128, TILE], F32, tag="oT0")
                outT1 = outT_pool.tile([128, TILE], F32, tag="oT1")
                outT2 = outT_pool.tile([32, TILE], F32, tag="oT2")
                nc.scalar.copy(out=outT0[:, :], in_=pA[:, :])
                nc.scalar.copy(out=outT1[:, :], in_=pB[:, :])
                nc.vector.tensor_copy(outT2[:, :], pC[:, :])

                osb = osb_pool.tile([128, NSUB * OUT_W], F32, tag="osb")
                osb_v = osb[:].rearrange("p (s j) -> p s j", s=NSUB)
                for s in range(NSUB):
                    pN = pN_pool.tile([128, OUT_W], F32, tag="pN")
                    nc.tensor.transpose(pN[:, 0:128],
                                        outT0[:, 128 * s:128 * (s + 1)],
                                        ident)
                    nc.tensor.transpose(pN[:, 128:256],
                                        outT1[:, 128 * s:128 * (s + 1)],
                                        ident)
                    nc.tensor.transpose(pN[:, 256:288],
                                        outT2[:, 128 * s:128 * (s + 1)],
                                        wsb_sb[0:32, IDENT_COL:IDENT_COL + 32])
                    nc.scalar.copy(out=osb_v[:, s, :], in_=pN[:, :])
                nc.sync.dma_start(
                    out=out_r[t].rearrange("s p j -> p s j"),
                    in_=osb_v[:, :, :],
                )
    nc.finalize()
    return nc


_NC_CACHE = {}


def _get_nc(nit=NIT):
    if nit not in _NC_CACHE:
        _NC_CACHE[nit] = build_nc(nit)
    return _NC_CACHE[nit]


def _make_in_maps(atom_agg, res_emb, w, b, backbone_idx, ca_res_idx, nit=NIT):
    atom_agg = np.ascontiguousarray(np.asarray(atom_agg, dtype=np.float32))
    res_emb = np.ascontiguousarray(np.asarray(res_emb, dtype=np.float32))
    backbone_idx = np.asarray(backbone_idx)
    ca_res_idx = np.asarray(ca_res_idx)
    num_res = res_emb.shape[0]
    assert num_res == R_TOTAL, f"kernel compiled for {R_TOTAL} residues"

    wsb = build_wsb(w, b)
    A = atom_agg.reshape(atom_agg.shape[0], CA_W)
    E = res_emb.reshape(num_res, RES_W)

    ca_atom = backbone_idx.reshape(-1, 4)[:, 1]
    fast = (
        ca_atom.shape[0] == num_res
        and np.array_equal(ca_res_idx, np.arange(num_res, dtype=ca_res_idx.dtype))
        and np.array_equal(ca_atom, 4 * np.arange(num_res, dtype=ca_atom.dtype) + 1)
    )
    cont = None
    if not fast:
        cont = np.zeros((num_res, CA_W), np.float32)
        cont[ca_res_idx] = A[ca_atom]

    rp = nit * TILE
    rs = min(RS, rp)
    in_maps = []
    for c in range(N_CORES):
        r0 = c * RS
        resS = np.zeros((rp, RES_W), np.float32)
        resS[:rs] = E[r0:r0 + rs]
        vatS = np.zeros((4 * rp, CA_W), np.float32)
        if fast:
            vatS[:4 * rs] = A[4 * r0:4 * r0 + 4 * rs]
        else:
            vatS[1:4 * rs:4] = cont[r0:r0 + rs]
        in_maps.append({"res": resS, "vat": vatS, "wsb": wsb})
    return in_maps


def _run(in_maps, trace=False, **kw):
    nc = _get_nc()
    return run_bass_kernel_spmd(nc, in_maps, core_ids=list(range(N_CORES)),
                                trace=trace, **kw)


def _gather_out(results):
    out = np.empty((R_TOTAL, NUM_COEF, NODE_C), np.float32)
    for c in range(N_CORES):
        out[c * RS:(c + 1) * RS] = \
            results[c]["out"][:RS].reshape(RS, NUM_COEF, NODE_C)
    return out


def kernel(atom_agg, res_emb, w, b, backbone_idx, ca_res_idx):
    in_maps = _make_in_maps(atom_agg, res_emb, w, b, backbone_idx, ca_res_idx)
    res = _run(in_maps, trace=False)
    return _gather_out(res.results)


def kernel_profiled(atom_agg, res_emb, w, b, backbone_idx, ca_res_idx, **kw):
    """Same as kernel() but requests an NTFF trace; returns (out, BassKernelResults)."""
    in_maps = _make_in_maps(atom_agg, res_emb, w, b, backbone_idx, ca_res_idx)
    res = _run(in_maps, trace=True, **kw)
    return _gather_out(res.results), res


def build_null_nc(nit=NIT):
    """Same I/O signature as build_nc but near-zero work — measures the
    per-call dispatch overhead so it can be subtracted."""
    nc = bacc.Bacc()
    rp = nit * TILE
    res_d = nc.declare_dram_parameter("res", [rp, RES_W], F32, isOutput=False)
    nc.declare_dram_parameter("vat", [4 * rp, CA_W], F32, isOutput=False)
    nc.declare_dram_parameter("wsb", [128, WSB_FULL], F32, isOutput=False)
    out_d = nc.declare_dram_parameter("out", [rp, OUT_W], F32, isOutput=True)
    with TileContext(nc) as tc:
        with tc.tile_pool(name="t", bufs=1) as pool:
            tl = pool.tile([128, RES_W], F32)
            nc.sync.dma_start(out=tl[:], in_=res_d[0:128, :])
            nc.sync.dma_start(out=out_d[0:128, :], in_=tl[:])
    nc.finalize()
    return nc


def _timed_fn(nc, n_loop):
    """Build jitted 8-core executor that runs the NEFF n_loop times per call."""
    import jax
    from concourse import bass2jax as B

    B.install_neuronx_cc_hook()
    partition_name = nc.partition_id_tensor.name if nc.partition_id_tensor else None
    in_names, out_names, out_avals, zero_outs = [], [], [], []
    import concourse.mybir as mb
    for alloc in nc.m.functions[0].allocations:
        if not isinstance(alloc, mb.MemoryLocationSet):
            continue
        name = alloc.memorylocations[0].name
        if alloc.kind == "ExternalInput":
            if name != partition_name:
                in_names.append(name)
        elif alloc.kind == "ExternalOutput":
            shape = tuple(alloc.tensor_shape)
            dtype = mb.dt.np(alloc.dtype)
            out_avals.append(jax.core.ShapedArray(shape, dtype))
            out_names.append(name)
            zero_outs.append(np.zeros(shape, dtype))
    n_params = len(in_names)
    in_names = in_names + out_names
    if partition_name is not None:
        in_names.append(partition_name)

    def _body(*args):
        operands = list(args)
        if partition_name is not None:
            operands.append(B.partition_id_tensor())
        return tuple(B._bass_exec_p.bind(
            *operands,
            out_avals=tuple(out_avals),
            in_names=tuple(in_names),
            out_names=tuple(out_names),
            lowering_input_output_aliases=(),
            sim_require_finite=True,
            sim_require_nnan=True,
            nc=nc,
        ))

    mesh = B.Mesh(np.asarray(jax.devices()[:N_CORES]), ("core",))
    spec = B.PartitionSpec("core")
    fn = jax.jit(
        B.shard_map(_body, mesh=mesh,
                    in_specs=(spec,) * (n_params + len(out_names)),
                    out_specs=(spec,) * len(out_names), check_rep=False),
        keep_unused=True,
    )
    return fn, mesh, n_params, in_names, zero_outs


def kernel_timed(atom_agg, res_emb, w, b, backbone_idx, ca_res_idx, reps=12):
    """Returns (out, per_exec_seconds, info). Times the NEFF with
    device-resident inputs and subtracts the dispatch overhead measured on a
    near-empty NEFF with the identical parameter signature."""
    import time

    import jax

    in_maps = _make_in_maps(atom_agg, res_emb, w, b, backbone_idx, ca_res_idx)

    def bench(nc):
        fn, mesh, n_params, in_names, zero_outs = _timed_fn(nc, 1)
        spec = jax.sharding.NamedSharding(mesh, jax.sharding.PartitionSpec("core"))
        per_core = [[np.asarray(m[n]) for n in in_names[:n_params]] for m in in_maps]
        concat = [np.concatenate([per_core[c][i] for c in range(N_CORES)], 0)
                  for i in range(n_params)]
        concat += [np.zeros((N_CORES * z.shape[0], *z.shape[1:]), z.dtype)
                   for z in zero_outs]
        din = [jax.device_put(x, spec) for x in concat]
        outs = fn(*din)
        jax.block_until_ready(outs)  # compile + warm
        ts = []
        for _ in range(reps):
            t0 = time.perf_counter()
            jax.block_until_ready(fn(*din))
            ts.append(time.perf_counter() - t0)
        return outs, ts

    outs, ts_main = bench(_get_nc())
    o = np.asarray(outs[0]).reshape(N_CORES, -1, OUT_W)
    out_np = np.concatenate(
        [o[c, :RS].reshape(RS, NUM_COEF, NODE_C) for c in range(N_CORES)], 0)
    _, ts_null = bench(build_null_nc())
    per_exec = min(ts_main) - min(ts_null)
    info = {"main_ms": sorted(t * 1e3 for t in ts_main)[:4],
            "null_ms": sorted(t * 1e3 for t in ts_null)[:4]}
    return out_np, per_exec, info


BUILDERS = {
    "v1_full": lambda: build_nc(NIT),
    "v1_n8": lambda: build_nc(8),
    "v1_n4": lambda: build_nc(4),
    "null_n4": lambda: build_null_nc(4),
    "null_n8": lambda: build_null_nc(8),
}

